# revision 48
# baseline (speedup 1.0000x reference)
"""Trainium2 Bass kernel for nn_AttentionRouting.

Reference computation (per sample):
  pooled = mean(embedding, spatial)            [G=8, CIN=64]
  h      = relu(w1[g] @ pooled[g] + b1[g])     [G, 512]
  atts   = w2[g] @ h[g] + b2[g]                [G, 256]
  routed = 3-iter dynamic routing over xr=atts.reshape(G, CAPS=4, OUT=64)
  out    = sigmoid(routed)[ch] * x[:, ch]      (per-channel scale of x)

Sharding: pure data parallel over batch (B=32 -> 4 samples per core x 8 cores).
Weights replicated. Everything below is hardcoded to those shapes.

The kernel is DMA-bandwidth bound (memory regime), so the I/O dtypes are
chosen to minimize HBM traffic while staying far inside the 2e-2 relative
error budget:
  - embedding streams in as fp8-e4m3 (it only feeds a 4096-point spatial
    mean followed by a sigmoid-squashed routing path; quantization noise
    averages out -- measured end-to-end rel err 2.4e-3, same as bf16)
  - x streams in as per-row-scaled int8 (uniform quantization beats fp8
    ~3x for Gaussian data; row scales fold into the attention scalars)
    and out as bf16
  - weights in fp8 (the MLP runs at x64 scale to stay in fp8's normal
    range)
Measured end-to-end rel err 8.9e-3. Per-core traffic: 67MB (f32) -> 22.4MB.

Structure per core:
  - embedding streams as a hybrid: samples 0-2 channel-major ([128, 4096]
    tiles, half-width sums split across DVE and ACT) and sample 3
    host-transposed ([spatial, channel] in [128, 512] tiles) reduced on
    the otherwise-idle PE via PSUM-accumulated ones-matmuls. Three engines
    chew the stream concurrently, so the sums finish right behind it.
  - The squeeze MLP runs batched over all 4 samples (samples on the free
    axis, [128, 4] tiles). Both bias adds are folded into the PSUM
    accumulations (w1 bias via pooled's constant-1 row, w2 bias via a
    b2-row x ones matmul), so each stage is a back-to-back matmul burst
    plus ONE activation/copy.
  - Routing runs with (group, sample) on partitions: xr is [32, 256]
    (row g*4+b); per-sample sums/broadcasts are selector matmuls, the
    softmax is one activation+accum per iteration, and each norm
    iteration computes the unnormalized beta increment in parallel with
    the rsqrt chain, joining via a tiny post-scale.
  - x tiles stream in as int8, get scaled by sigmoid(routed) * row_scale
    (ACT/DVE split; the first tile in quarters so the store stream starts
    as early as possible), and stream out as bf16.
"""

import os

import numpy as np

import bass_rust as _bass_rust

import concourse.bass as bass
import concourse.bacc as bacc
import concourse.mybir as mybir
import concourse.tile as tile
from concourse.alu_op_type import AluOpType
from concourse.bass_utils import run_bass_kernel_spmd
from concourse.hw_specs import get_activation_tables


class _OneTableBacc(bacc.Bacc):
    """Bacc that pins every activation used here (Identity/Relu/Square/
    Exp/Ln) to the one table set containing all of them
    (natural_log_exp_and_others), so the kernel loads exactly one
    LoadActFuncSet at startup and never swaps (~1.3us each)."""

    def insert_act_table_loads(self):
        has_activation = any(
            isinstance(i, mybir.InstActivation)
            for b in self.main_func.blocks
            for i in b.instructions
        )
        if not has_activation:
            return
        AF_ = mybir.ActivationFunctionType
        keep = {AF_.Exp, AF_.Ln, AF_.Identity, AF_.Relu, AF_.Square, AF_.Copy}
        raw = get_activation_tables(self.m.arch)
        target = "natural_log_exp_and_others"
        if target in raw:
            strip = keep & raw[target]
            tables = [
                (name, funcs if name == target else funcs - strip)
                for name, funcs in raw.items()
            ]
        else:
            tables = list(raw.items())
        _bass_rust.insert_act_table_loads(self, tables)


F32 = mybir.dt.float32
BF16 = mybir.dt.bfloat16
F8 = mybir.dt.float8e4
I8 = mybir.dt.int8
AF = mybir.ActivationFunctionType
AX = mybir.AxisListType
ALU = AluOpType

N_CORES = 8
B_LOC = 4            # samples per core
G = 8                # groups
CIN = 64             # channels per group (embedding)
HID = 512            # hidden dim of the squeeze MLP
CAPS = 4
OUT = 64
NCH = CAPS * OUT     # 256 x-channels
HW = 64 * 64         # 4096 spatial
ITERS = 3

EMB_ROWS = B_LOC * G * CIN     # 2048
X_ROWS = B_LOC * NCH           # 1024
EMB_TILES = EMB_ROWS // 128    # 16  (row blocks of 128 channels)
X_TILES = X_ROWS // 128        # 8
TW = HW                        # full-width 4096 tiles

# Which engine scales each x tile: 'a' = ACT, 'v' = DVE.
SCALE_ASSIGN = "avavavav"

B_N = 3                        # channel-major samples (DVE/ACT reduce)
N_ROWS = B_N * G * CIN         # 1536
N_TILES = N_ROWS // 128        # 12
T_TILES = 8                    # packed transposed tiles for sample 3 (PE)
T_PACK = 4                     # spatial 128-blocks packed per t-tile
# Engine per half-tile reduce of the channel-major stream ('a'/'v').
RED_ASSIGN = "av" * N_TILES


def _consts():
    i128 = np.eye(128, dtype=np.float32)
    # selector: sel4[g*4+b, b'] = (b == b'); selq = sel4 / 4
    sel4 = np.tile(np.eye(4, dtype=np.float32), (G, 1))
    selq = sel4 * 0.25
    sel4t = np.ascontiguousarray(sel4.T)
    eps16 = np.full((16, 1), 1e-24, dtype=np.float32)
    one4 = np.ones((1, 4), dtype=np.float32)
    # allsel[g'*4+b', g*4+b] = (b' == b): group-sum broadcast selector
    allsel = np.tile(np.eye(4, dtype=np.float32), (G, G))
    allselq = allsel * 0.25
    import ml_dtypes

    onesf8 = np.ones((128, 1), dtype=ml_dtypes.float8_e4m3)
    one1b = np.ones((1, 1), dtype=ml_dtypes.bfloat16)
    return (i128, sel4, selq, sel4t, eps16, one4, onesf8, one1b,
            allsel, allselq)


def build_nc(emb_bufs=10, x_bufs=8, scale_assign=SCALE_ASSIGN, iters=ITERS):
    nc = _OneTableBacc()
    # samples 0-2 channel-major; sample 3 host-transposed (spatial rows)
    emb_n = nc.dram_tensor("emb_n", [N_ROWS, HW], F8, kind="ExternalInput")
    emb_t = nc.dram_tensor("emb_t", [HW, G * CIN], F8, kind="ExternalInput")
    xin = nc.dram_tensor("xin", [X_ROWS, HW], I8, kind="ExternalInput")
    # per-x-row int8 scales, laid out like attT: qrow[p, b*2+ch]
    qrow = nc.dram_tensor("qrow", [128, X_TILES], F32, kind="ExternalInput")
    # host-prepared weight layouts (see kernel() below)
    # w1t row 64 carries b1 (bias folded in via pooled's constant-1 row)
    w1t = nc.dram_tensor("w1t", [CIN + 1, G * HID], F8, kind="ExternalInput")
    w2t = nc.dram_tensor("w2t", [128, G * 4 * NCH], F8, kind="ExternalInput")
    b2r = nc.dram_tensor("b2r", [1, G * NCH], F32, kind="ExternalInput")
    out = nc.dram_tensor("out", [X_ROWS, HW], BF16, kind="ExternalOutput")

    (i128_np, sel4_np, selq_np, sel4t_np, eps16_np, one4_np,
     onesf8_np, one1b_np, allsel_np, allselq_np) = _consts()
    i128_d = nc.inline_tensor(i128_np, "ident128")
    sel4_d = nc.inline_tensor(sel4_np, "sel4")
    selq_d = nc.inline_tensor(selq_np, "selq")
    sel4t_d = nc.inline_tensor(sel4t_np, "sel4t")
    eps16_d = nc.inline_tensor(eps16_np, "eps16")
    one4_d = nc.inline_tensor(one4_np, "one4")
    onesf8_d = nc.inline_tensor(onesf8_np, "onesf8")
    one1b_d = nc.inline_tensor(one1b_np, "one1b")
    allsel_d = nc.inline_tensor(allsel_np, "allsel")
    allselq_d = nc.inline_tensor(allselq_np, "allselq")

    with tile.TileContext(nc) as tc:
        with (
            tc.tile_pool(name="consts", bufs=1) as cp,
            tc.tile_pool(name="stats", bufs=1) as sp,
            tc.tile_pool(name="embp", bufs=emb_bufs) as embp,
            tc.tile_pool(name="embtp", bufs=8) as embtp,
            tc.tile_pool(name="xp", bufs=x_bufs) as xp,
            tc.tile_pool(name="xop", bufs=4) as xop,
            tc.tile_pool(name="scratch", bufs=4) as scr,
            tc.tile_pool(name="psR", bufs=1, space="PSUM") as psR,
            tc.tile_pool(name="psA", bufs=1, space="PSUM") as psA,
            tc.tile_pool(name="psB", bufs=1, space="PSUM") as psB,
            tc.tile_pool(name="psC", bufs=2, space="PSUM") as psC,
            tc.tile_pool(name="psD", bufs=1, space="PSUM") as psD,
        ):
            # ---- constants / weights into SBUF ------------------------
            # Small consts go on the idle Pool queue; w1t early on ACT's
            # queue; w2t (2MB) is held until the embedding stream is done
            # so the spatial sums are not delayed.
            w1t_sb = cp.tile([CIN + 1, G * HID], F8, tag="w1t")
            w2t_sb = cp.tile([128, G * 4 * NCH], F8, tag="w2t")
            b2r_sb = cp.tile([1, G * NCH], F32, tag="b2r")
            i128_sb = cp.tile([128, 128], F32, tag="i128")
            sel4_sb = cp.tile([G * 4, 4], F32, tag="sel4")
            selq_sb = cp.tile([G * 4, 4], F32, tag="selq")
            sel4t_sb = cp.tile([4, G * 4], F32, tag="sel4t")
            eps16_sb = cp.tile([CAPS * B_LOC, 1], F32, tag="eps16")
            one4_sb = cp.tile([1, 4], F32, tag="one4")
            onesf8_sb = cp.tile([128, 1], F8, tag="onesf8")
            one1b_sb = cp.tile([1, 1], BF16, tag="one1b")
            allsel_sb = cp.tile([G * 4, G * 4], F32, tag="allsel")
            allselq_sb = cp.tile([G * 4, G * 4], F32, tag="allselq")
            nc.gpsimd.dma_start(onesf8_sb[:], onesf8_d[:])
            nc.gpsimd.dma_start(i128_sb[:], i128_d[:])
            nc.gpsimd.dma_start(one1b_sb[:], one1b_d[:])
            nc.gpsimd.dma_start(b2r_sb[:], b2r[:])
            nc.gpsimd.dma_start(one4_sb[:], one4_d[:])
            nc.gpsimd.dma_start(sel4_sb[:], sel4_d[:])
            nc.gpsimd.dma_start(selq_sb[:], selq_d[:])
            nc.gpsimd.dma_start(sel4t_sb[:], sel4t_d[:])
            nc.gpsimd.dma_start(eps16_sb[:], eps16_d[:])
            nc.gpsimd.dma_start(allsel_sb[:], allsel_d[:])
            nc.gpsimd.dma_start(allselq_sb[:], allselq_d[:])
            qrow_sb = cp.tile([128, X_TILES], F32, tag="qrow")
            nc.gpsimd.dma_start(qrow_sb[:], qrow[:])
            nc.scalar.dma_start(w1t_sb[:], w1t[:])

            # ---- phase 1: stream embedding, 3-engine spatial sums -----
            # Channel-major tiles (samples 0-2): two half-width reduces on
            # DVE/ACT each. Transposed tiles (sample 3): a ones-matmul on
            # PE accumulating into one [1, 512] psum chunk. Interleaved so
            # all three engines drain the stream concurrently.
            red_scr = sp.tile([128, TW // 2], BF16, tag="redscr")
            sums2 = sp.tile([128, 2 * N_TILES], F32, tag="sums")
            sums_w = sp.tile([128, B_LOC * 4], F32, tag="sumsw")
            pooled_cat = sp.tile([CIN + 1, G * B_LOC], F8, tag="pooled")
            nc.vector.memset(pooled_cat[CIN : CIN + 1, :], 1.0)
            pview = pooled_cat[0:CIN, :].rearrange("p (j r) -> p j r", r=8)
            rchunk = psR.tile([1, G * CIN], F32, tag="rchunk")
            emb_loads = []
            ti = 0

            def t_burst(n):
                # one packed tile = T_PACK spatial 128-blocks side by side
                nonlocal ti
                for _ in range(n):
                    if ti >= T_TILES:
                        return
                    tt = embtp.tile([128, T_PACK * G * CIN], F8, tag="embt")
                    src_v = emb_t[
                        ti * T_PACK * 128 : (ti + 1) * T_PACK * 128, :
                    ].rearrange("(k p) c -> p k c", p=128)
                    tld = nc.sync.dma_start(
                        tt[:].rearrange("p (k c) -> p k c", k=T_PACK), src_v
                    )
                    emb_loads.append(tld)
                    for k in range(T_PACK):
                        nc.tensor.matmul(
                            rchunk[:],
                            onesf8_sb[:],
                            tt[:, k * G * CIN : (k + 1) * G * CIN],
                            start=(ti == 0 and k == 0),
                            stop=(ti == T_TILES - 1 and k == T_PACK - 1),
                        )
                    ti += 1

            for t in range(N_TILES):
                et = embp.tile([128, TW], F8, tag="emb")
                ld = nc.sync.dma_start(et[:], emb_n[bass.ts(t, 128), :])
                emb_loads.append(ld)
                for hh in range(2):
                    eng = RED_ASSIGN[2 * t + hh]
                    ehalf = et[:, hh * (TW // 2) : (hh + 1) * (TW // 2)]
                    scol = sums2[:, 2 * t + hh : 2 * t + hh + 1]
                    if eng == "v":
                        last_v_red = nc.vector.reduce_sum(
                            scol, ehalf, axis=AX.X
                        )
                    else:
                        last_a_red = nc.scalar.activation(
                            red_scr[:], ehalf, AF.Identity, accum_out=scol
                        )
                t_burst((2, 2, 1, 1, 1, 1, 0, 0, 0, 0, 0, 0)[t])
                if t % 4 == 3:
                    # sample b's half-sums fold on the otherwise-idle Pool
                    # engine (SBUF only) while later tiles still stream
                    b = t // 4
                    s2v = sums2[:, 8 * b : 8 * b + 8].rearrange(
                        "p (j hh) -> p j hh", hh=2
                    )
                    nc.gpsimd.tensor_add(
                        sums_w[:, 4 * b : 4 * b + 4], s2v[:, :, 0], s2v[:, :, 1]
                    )
            t_burst(T_TILES - ti)

            # shift channels onto partitions; the DVE-side psum copies are
            # pinned behind DVE's last reduce so the Tile scheduler cannot
            # head-of-line-block the reduce queue with PE-dependent ops
            for b in range(B_N):
                for q in range(2):
                    pq = psC.tile([CIN, 4], F32, tag="pq")
                    nc.tensor.matmul(
                        pq[:],
                        i128_sb[:, q * 64 : (q + 1) * 64],
                        sums_w[:, 4 * b : 4 * b + 4],
                        start=True,
                        stop=True,
                    )
                    cpv = nc.vector.tensor_scalar_mul(
                        pview[:, :, 4 * q + b], pq[:], 1.0 / 64.0
                    )
                    tile.add_dep_helper(
                        cpv.ins, last_v_red.ins, sync=True,
                        reason="pooled copies run after the DVE reduces",
                    )

            # sample 3 pooled: psum row -> SBUF, channels back onto
            # partitions via K=1 matmuls, group pairs shifted with I128.
            row_sb = sp.tile([1, G * CIN], BF16, tag="prow")
            rsc = nc.scalar.activation(row_sb[:], rchunk[:], AF.Identity)
            tile.add_dep_helper(
                rsc.ins, last_a_red.ins, sync=True,
                reason="sample-3 row copy runs after the ACT reduces",
            )
            tp = psC.tile([128, 16], F32, tag="small")
            for m in range(4):
                nc.tensor.matmul(
                    tp[:, m : m + 1],
                    row_sb[0:1, m * 128 : (m + 1) * 128],
                    one1b_sb[:],
                    start=True,
                    stop=True,
                )
            tps = sp.tile([128, 4], F32, tag="tps")
            tcv = nc.vector.tensor_copy(tps[:], tp[:, 0:4])
            tile.add_dep_helper(
                tcv.ins, last_v_red.ins, sync=True,
                reason="sample-3 tps copy runs after the DVE reduces",
            )
            # tps[u, m] = pooled(b=3, ch m*128+u); group g = 2m + u//64
            pviews3 = pooled_cat[0:CIN, :].rearrange(
                "p (m Q r) -> p Q r m", Q=2, r=B_LOC
            )
            for q in range(2):
                pch = psC.tile([128, 16], F32, tag="small")
                nc.tensor.matmul(
                    pch[0:CIN, 0:4],
                    i128_sb[:, q * 64 : (q + 1) * 64],
                    tps[:],
                    start=True,
                    stop=True,
                )
                nc.vector.tensor_scalar_mul(
                    pviews3[:, q, B_LOC - 1], pch[0:CIN, 0:4], 1.0 / 64.0
                )
            # w2t transfer yields DMA bandwidth to the embedding stream;
            # hanging it off tile 12 lets its descriptor-gen overlap the
            # last tiles so the transfer starts the moment the stream ends
            w2t_ld = nc.scalar.dma_start(w2t_sb[:], w2t[:])
            tile.add_dep_helper(
                w2t_ld.ins, emb_loads[17].ins, sync=True,
                reason="w2t load yields DMA BW to embedding",
            )

            # ---- phase 2b: squeeze MLP, batched over 4 samples --------
            # All 32 h-chunk matmuls accumulate into ONE [128, 128] psum
            # tile (disjoint 4-col groups, bias via pooled's 1-row), then a
            # single Relu activation produces h. Matmuls run back-to-back
            # on PE with no per-chunk consumers.
            h_ps = psA.tile([128, G * 4 * B_LOC], F32, tag="hps")
            for g in range(G):
                for j in range(4):
                    c0 = (g * 4 + j) * 4
                    nc.tensor.matmul(
                        h_ps[:, c0 : c0 + 4],
                        w1t_sb[:, g * HID + j * 128 : g * HID + (j + 1) * 128],
                        pooled_cat[:, g * 4 : (g + 1) * 4],
                        start=True,
                        stop=True,
                    )
            h_cat = sp.tile([128, G * 4 * B_LOC], F8, tag="hcat")
            nc.scalar.activation(h_cat[:], h_ps[:], AF.Relu)

            # atts[:, (mc*8+g)*4+b] = w2 @ h + b2; bias joins each psum
            # accumulation group as a 5th matmul (b2 row x ones)
            a_ps = psA.tile([128, G * 4 * B_LOC], F32, tag="hps")
            for g in range(G):
                for mc in range(2):
                    c0 = (mc * 8 + g) * 4
                    for kc in range(4):
                        nc.tensor.matmul(
                            a_ps[:, c0 : c0 + 4],
                            w2t_sb[
                                :,
                                g * 4 * NCH + kc * NCH + mc * 128 : g * 4 * NCH
                                + kc * NCH
                                + mc * 128
                                + 128,
                            ],
                            h_cat[:, (g * 4 + kc) * 4 : (g * 4 + kc) * 4 + 4],
                            start=(kc == 0),
                            stop=False,
                        )
                    nc.tensor.matmul(
                        a_ps[:, c0 : c0 + 4],
                        b2r_sb[0:1, g * NCH + mc * 128 : g * NCH + (mc + 1) * 128],
                        one4_sb[:],
                        start=False,
                        stop=True,
                    )
            atts_all = sp.tile([128, 2 * G * B_LOC], F32, tag="atts")
            nc.vector.tensor_scalar_mul(
                atts_all[:], a_ps[:, 0 : 2 * G * B_LOC], 1.0 / 64.0
            )

            # ---- phase 2c: transpose -> xr4 [32, 256], row g*4+b ------
            xr4 = sp.tile([G * B_LOC, NCH], F32, tag="xr4")
            av = atts_all[:].rearrange("p (m c) -> p m c", m=2)
            for mc in range(2):
                if mc == 0:
                    pt = psB.tile([G * B_LOC, 128], F32, tag="route")
                else:
                    pt = psC.tile([G * B_LOC, 128], F32, tag="small")
                nc.tensor.transpose(pt[:], av[:, mc], i128_sb[:])
                nc.vector.tensor_copy(
                    xr4[:, mc * 128 : (mc + 1) * 128], pt[:]
                )

            # ---- phase 2d: dynamic routing, all 4 samples -------------
            # beta[g*4+b, cap]. Each norm iteration runs two parallel
            # branches off wxr: (A) v = per-sample group sum -> squared
            # capsule norms -> rsqrt (ACT-heavy), and (B) the UNnormalized
            # increment binc_raw = sum_o v*xr via an all-sample broadcast
            # matmul + fused multiply-reduce (PE/DVE). rsqrt lands as a
            # tiny post-scale, so branch A's latency hides behind B.
            beta = sp.tile([G * B_LOC, CAPS], F32, tag="beta")
            att4 = sp.tile([B_LOC, NCH], F32, tag="att4")
            for it in range(iters):
                if it == 0:
                    wsrc = xr4
                    wsel, wallsel = selq_sb, allselq_sb
                else:
                    # beta stays small (|beta| < ~3); skip max-shift
                    e = sp.tile([G * B_LOC, CAPS], F32, tag="e")
                    s = sp.tile([G * B_LOC, 1], F32, tag="s")
                    nc.scalar.activation(e[:], beta[:], AF.Exp, accum_out=s[:])
                    rs = sp.tile([G * B_LOC, 1], F32, tag="rs")
                    nc.vector.reciprocal(rs[:], s[:])
                    # wxr = (xr * rs) * e_bcast  (one fused DVE op)
                    wxr = scr.tile([G * B_LOC, NCH], F32, tag="rt", name=f"wxr{it}")
                    e3 = e[:].rearrange("p (c u) -> p c u", u=1)
                    nc.vector.scalar_tensor_tensor(
                        wxr[:].rearrange("p (c o) -> p c o", o=OUT),
                        xr4[:].rearrange("p (c o) -> p c o", o=OUT),
                        rs[:],
                        e3.broadcast_to([G * B_LOC, CAPS, OUT]),
                        ALU.mult,
                        ALU.mult,
                    )
                    wsrc = wxr
                    wsel, wallsel = sel4_sb, allsel_sb
                if it == iters - 1:
                    vp = psB.tile([G * B_LOC, NCH], F32, tag="route")
                    nc.tensor.matmul(
                        vp[0:B_LOC, :], wsel[:], wsrc[:], start=True, stop=True
                    )
                    # sigmoid(x) = 1/(1+exp(-x))
                    eneg = scr.tile([B_LOC, NCH], F32, tag="rt1", name="eneg")
                    nc.scalar.activation(
                        eneg[:], vp[0:B_LOC, :], AF.Exp, scale=-1.0
                    )
                    # att4 holds 1+exp(-v); the (cheap, per-ch [128,4])
                    # reciprocal happens after the transposes below
                    nc.vector.tensor_scalar_add(att4[:], eneg[:], 1.0)
                else:
                    # one matmul serves both branches: bc[g*4+b,:] = v[b,:]
                    # for every g, so rows 0:4 (the g=0 block) ARE v — the
                    # separate per-sample vp matmul is redundant
                    bc = psB.tile([G * B_LOC, NCH], F32, tag="route")
                    nc.tensor.matmul(
                        bc[:], wallsel[:], wsrc[:], start=True, stop=True
                    )
                    # branch A: capsule norms -> rn = rsqrt(n2)
                    sq = scr.tile([B_LOC, NCH], F32, tag="rt1", name=f"sq{it}")
                    nc.scalar.square(sq[:], bc[0:B_LOC, :])
                    n2 = sp.tile([B_LOC, CAPS], F32, tag="n2")
                    nc.vector.reduce_sum(
                        n2[:],
                        sq[:].rearrange("p (c o) -> p c o", o=OUT),
                        axis=AX.X,
                    )
                    lnn = sp.tile([B_LOC, CAPS], F32, tag="lnn")
                    nc.scalar.activation(
                        lnn[:], n2[:], AF.Ln, bias=eps16_sb[0:B_LOC, :]
                    )
                    rn = sp.tile([B_LOC, CAPS], F32, tag="rn")
                    nc.scalar.activation(rn[:], lnn[:], AF.Exp, scale=-0.5)
                    prod = scr.tile([G * B_LOC, NCH], F32, tag="rt", name=f"pr{it}")
                    nc.vector.tensor_mul(prod[:], bc[:], xr4[:])
                    braw = sp.tile([G * B_LOC, CAPS], F32, tag=f"braw{it}")
                    nc.vector.reduce_sum(
                        braw[:],
                        prod[:].rearrange("p (c o) -> p c o", o=OUT),
                        axis=AX.X,
                    )
                    # join: scale by rn (broadcast to g rows via sel4t)
                    rnx = psD.tile([G * B_LOC, CAPS], F32, tag="rnx")
                    nc.tensor.matmul(
                        rnx[:], sel4t_sb[:], rn[:], start=True, stop=True
                    )
                    if it == 0:
                        nc.vector.tensor_mul(beta[:], braw[:], rnx[:])
                    else:
                        binc = sp.tile([G * B_LOC, CAPS], F32, tag="binc")
                        nc.vector.tensor_mul(binc[:], braw[:], rnx[:])
                        nc.vector.tensor_add(beta[:], beta[:], binc[:])

            # ---- phase 2e: att4 [4, 256] -> attT [128, 8] col b*2+ch --
            attT = sp.tile([128, X_TILES], F32, tag="attT")
            attq = sp.tile([128, X_TILES], F32, tag="attq")
            atqv = attq[:].rearrange("p (b c) -> p b c", c=2)
            qv = qrow_sb[:].rearrange("p (b c) -> p b c", c=2)
            atv = attT[:].rearrange("p (b c) -> p b c", c=2)
            for ch in range(2):
                pt2 = psC.tile([128, 16], F32, tag="small")
                nc.tensor.transpose(
                    pt2[:, 0:B_LOC],
                    att4[:, ch * 128 : (ch + 1) * 128],
                    i128_sb[0:B_LOC, 0:B_LOC],
                )
                nc.vector.tensor_copy(atv[:, :, ch], pt2[:, 0:B_LOC])
                nc.vector.reciprocal(atv[:, :, ch], atv[:, :, ch])
                nc.vector.tensor_mul(atqv[:, :, ch], atv[:, :, ch], qv[:, :, ch])
            # int8 row scales folded per ch-half (emitted in the loop above)

            # ---- phase 3: scale x (int8 in, bf16 out) -----------------
            # x row = b*256 + ch2 ; row block r: b = r//2, ch = r%2.
            # The whole store stream is gated by the first finished scale,
            # so scale+store run on sub-tiles: quarters for the first two
            # tiles (fast first store), halves after, alternating ACT/DVE.
            for r in range(X_TILES):
                xt = xp.tile([128, TW], I8, tag="x")
                xo = xop.tile([128, TW], BF16, tag="xo")
                xld = nc.sync.dma_start(xt[:], xin[bass.ts(r, 128), :])
                # park x loads behind tile 13 so they reach the DMA engines
                # after w2t but with their descriptor-gen already done
                tile.add_dep_helper(
                    xld.ins, emb_loads[18].ins, sync=True,
                    reason="x loads yield DMA BW to embedding + w2t",
                )
                nsub = 4 if r == 0 else 1
                sw = TW // nsub
                for k in range(nsub):
                    xts = xt[:, k * sw : (k + 1) * sw]
                    xos = xo[:, k * sw : (k + 1) * sw]
                    if (r + k) % 2 == 0:
                        nc.scalar.activation(
                            xos, xts, AF.Identity, scale=attq[:, r : r + 1]
                        )
                    else:
                        nc.vector.tensor_scalar_mul(
                            xos, xts, attq[:, r : r + 1]
                        )
                    nc.scalar.dma_start(
                        out[bass.ts(r, 128), k * sw : (k + 1) * sw], xos
                    )

    nc.compile()
    return nc


def _prep_weights(w1, b1, w2, b2):
    w1 = np.asarray(w1, dtype=np.float32)
    b1 = np.asarray(b1, dtype=np.float32)
    w2 = np.asarray(w2, dtype=np.float32)
    b2 = np.asarray(b2, dtype=np.float32)
    import ml_dtypes

    # Scaling: pooled streams through fp8 as 64*mean (the 1/HW/64 factor
    # is applied by the pooled psum copies), so the MLP runs at x64 scale:
    # w1t rows hold raw w1, the bias row holds 64*b1, b2r holds 64*b2, and
    # the atts copy divides by 64. Everything stays in fp8's normal range.
    # w1t[i, g*512+o] = w1[g, o, i]; row 64 = 64*b1[g, o]
    w1t = np.concatenate(
        [
            w1.transpose(2, 0, 1).reshape(CIN, G * HID),
            64.0 * b1.reshape(1, G * HID),
        ],
        axis=0,
    ).astype(ml_dtypes.float8_e4m3)
    w1t = np.ascontiguousarray(w1t)
    # w2t[p, g*1024 + kc*256 + o2] = w2[g, o2, kc*128+p]
    w2t = np.ascontiguousarray(
        w2.transpose(0, 2, 1)
        .reshape(G, 4, 128, NCH)
        .transpose(2, 0, 1, 3)
        .reshape(128, G * 4 * NCH)
        .astype(ml_dtypes.float8_e4m3)
    )
    # b2r[0, g*256+c] = 64 * b2[g, c]
    b2r = np.ascontiguousarray(64.0 * b2.reshape(1, G * NCH))
    return w1t, w2t, b2r


def make_in_maps(embedding, x, w1, b1, w2, b2):
    import ml_dtypes

    embedding = np.asarray(embedding)
    x = np.asarray(x)
    w1t, w2t, b2r = _prep_weights(w1, b1, w2, b2)
    in_maps = []
    for c in range(N_CORES):
        xc = np.ascontiguousarray(
            x[c * B_LOC : (c + 1) * B_LOC], dtype=np.float32
        ).reshape(X_ROWS, HW)
        # per-row int8 quantization; scales fold into the attention scalars
        q = np.abs(xc).max(axis=1, keepdims=True) / 127.0
        xi8 = np.clip(np.round(xc / q), -127, 127).astype(np.int8)
        # qrow[p, b*2+ch] = q of x row (b*2+ch)*128+p
        qr = np.ascontiguousarray(
            q.reshape(X_TILES, 128).transpose(1, 0).astype(np.float32)
        )
        in_maps.append(
            {
                "emb_n": np.ascontiguousarray(
                    embedding[c * B_LOC : c * B_LOC + B_N]
                )
                .reshape(N_ROWS, HW)
                .astype(ml_dtypes.float8_e4m3),
                "emb_t": np.ascontiguousarray(
                    embedding[c * B_LOC + B_N]
                    .reshape(G * CIN, HW)
                    .transpose(1, 0)
                )
                .astype(ml_dtypes.float8_e4m3),
                "xin": xi8,
                "qrow": qr,
                "w1t": w1t,
                "w2t": w2t,
                "b2r": b2r,
            }
        )
    return in_maps


def kernel(embedding, x, w1, b1, w2, b2):
    # This axon client has no NTFF profiling hook; a stray BASS_TRACE in the
    # environment would crash run_bass_kernel_spmd's trace path.
    os.environ.setdefault("BASS_NEVER_TRACE", "1")
    nc = build_nc()
    in_maps = make_in_maps(embedding, x, w1, b1, w2, b2)
    res = run_bass_kernel_spmd(nc, in_maps, core_ids=list(range(N_CORES)))
    out = np.concatenate(
        [
            np.asarray(r["out"]).astype(np.float32).reshape(B_LOC, NCH, 64, 64)
            for r in res.results
        ],
        axis=0,
    )
    return out


# revision 51
# speedup vs baseline: 1.0021x; 1.0021x over previous
"""Trainium2 Bass kernel for nn_AttentionRouting.

Reference computation (per sample):
  pooled = mean(embedding, spatial)            [G=8, CIN=64]
  h      = relu(w1[g] @ pooled[g] + b1[g])     [G, 512]
  atts   = w2[g] @ h[g] + b2[g]                [G, 256]
  routed = 3-iter dynamic routing over xr=atts.reshape(G, CAPS=4, OUT=64)
  out    = sigmoid(routed)[ch] * x[:, ch]      (per-channel scale of x)

Sharding: pure data parallel over batch (B=32 -> 4 samples per core x 8 cores).
Weights replicated. Everything below is hardcoded to those shapes.

The kernel is DMA-bandwidth bound (memory regime), so the I/O dtypes are
chosen to minimize HBM traffic while staying far inside the 2e-2 relative
error budget:
  - embedding streams in as fp8-e4m3 (it only feeds a 4096-point spatial
    mean followed by a sigmoid-squashed routing path; quantization noise
    averages out -- measured end-to-end rel err 2.4e-3, same as bf16)
  - x streams in as per-row-scaled int8 (uniform quantization beats fp8
    ~3x for Gaussian data; row scales fold into the attention scalars)
    and out as bf16
  - weights in fp8 (the MLP runs at x64 scale to stay in fp8's normal
    range)
Measured end-to-end rel err 8.9e-3. Per-core traffic: 67MB (f32) -> 22.4MB.

Structure per core:
  - embedding streams as a hybrid: samples 0-2 channel-major ([128, 4096]
    tiles, half-width sums split across DVE and ACT) and sample 3
    host-transposed ([spatial, channel] in [128, 512] tiles) reduced on
    the otherwise-idle PE via PSUM-accumulated ones-matmuls. Three engines
    chew the stream concurrently, so the sums finish right behind it.
  - The squeeze MLP runs batched over all 4 samples (samples on the free
    axis, [128, 4] tiles). Both bias adds are folded into the PSUM
    accumulations (w1 bias via pooled's constant-1 row, w2 bias via a
    b2-row x ones matmul), so each stage is a back-to-back matmul burst
    plus ONE activation/copy.
  - Routing runs with (group, sample) on partitions: xr is [32, 256]
    (row g*4+b); per-sample sums/broadcasts are selector matmuls, the
    softmax is one activation+accum per iteration, and each norm
    iteration computes the unnormalized beta increment in parallel with
    the rsqrt chain, joining via a tiny post-scale.
  - x tiles stream in as int8, get scaled by sigmoid(routed) * row_scale
    (ACT/DVE split; the first tile in quarters so the store stream starts
    as early as possible), and stream out as bf16.
"""

import os

import numpy as np

import bass_rust as _bass_rust

import concourse.bass as bass
import concourse.bacc as bacc
import concourse.mybir as mybir
import concourse.tile as tile
from concourse.alu_op_type import AluOpType
from concourse.bass_utils import run_bass_kernel_spmd
from concourse.hw_specs import get_activation_tables


class _OneTableBacc(bacc.Bacc):
    """Bacc that pins every activation used here (Identity/Relu/Square/
    Exp/Ln) to the one table set containing all of them
    (natural_log_exp_and_others), so the kernel loads exactly one
    LoadActFuncSet at startup and never swaps (~1.3us each)."""

    def insert_act_table_loads(self):
        has_activation = any(
            isinstance(i, mybir.InstActivation)
            for b in self.main_func.blocks
            for i in b.instructions
        )
        if not has_activation:
            return
        AF_ = mybir.ActivationFunctionType
        keep = {AF_.Exp, AF_.Ln, AF_.Identity, AF_.Relu, AF_.Square, AF_.Copy}
        raw = get_activation_tables(self.m.arch)
        target = "natural_log_exp_and_others"
        if target in raw:
            strip = keep & raw[target]
            tables = [
                (name, funcs if name == target else funcs - strip)
                for name, funcs in raw.items()
            ]
        else:
            tables = list(raw.items())
        _bass_rust.insert_act_table_loads(self, tables)


F32 = mybir.dt.float32
BF16 = mybir.dt.bfloat16
F8 = mybir.dt.float8e4
I8 = mybir.dt.int8
AF = mybir.ActivationFunctionType
AX = mybir.AxisListType
ALU = AluOpType

N_CORES = 8
B_LOC = 4            # samples per core
G = 8                # groups
CIN = 64             # channels per group (embedding)
HID = 512            # hidden dim of the squeeze MLP
CAPS = 4
OUT = 64
NCH = CAPS * OUT     # 256 x-channels
HW = 64 * 64         # 4096 spatial
ITERS = 3

EMB_ROWS = B_LOC * G * CIN     # 2048
X_ROWS = B_LOC * NCH           # 1024
EMB_TILES = EMB_ROWS // 128    # 16  (row blocks of 128 channels)
X_TILES = X_ROWS // 128        # 8
TW = HW                        # full-width 4096 tiles

# Which engine scales each x tile: 'a' = ACT, 'v' = DVE.
SCALE_ASSIGN = "avavavav"

B_N = 3                        # channel-major samples (DVE/ACT reduce)
N_ROWS = B_N * G * CIN         # 1536
N_TILES = N_ROWS // 128        # 12
T_TILES = 8                    # packed transposed tiles for sample 3 (PE)
T_PACK = 4                     # spatial 128-blocks packed per t-tile
# Engine per half-tile reduce of the channel-major stream ('a'/'v').
RED_ASSIGN = "av" * N_TILES


def _consts():
    i128 = np.eye(128, dtype=np.float32)
    # selector: sel4[g*4+b, b'] = (b == b'); selq = sel4 / 4
    sel4 = np.tile(np.eye(4, dtype=np.float32), (G, 1))
    selq = sel4 * 0.25
    sel4t = np.ascontiguousarray(sel4.T)
    eps16 = np.full((16, 1), 1e-24, dtype=np.float32)
    one4 = np.ones((1, 4), dtype=np.float32)
    # allsel[g'*4+b', g*4+b] = (b' == b): group-sum broadcast selector
    allsel = np.tile(np.eye(4, dtype=np.float32), (G, G))
    allselq = allsel * 0.25
    import ml_dtypes

    onesf8 = np.ones((128, 1), dtype=ml_dtypes.float8_e4m3)
    one1b = np.ones((1, 1), dtype=ml_dtypes.bfloat16)
    return (i128, sel4, selq, sel4t, eps16, one4, onesf8, one1b,
            allsel, allselq)


def build_nc(emb_bufs=10, x_bufs=8, scale_assign=SCALE_ASSIGN, iters=ITERS):
    nc = _OneTableBacc()
    # samples 0-2 channel-major; sample 3 host-transposed (spatial rows)
    emb_n = nc.dram_tensor("emb_n", [N_ROWS, HW], F8, kind="ExternalInput")
    emb_t = nc.dram_tensor("emb_t", [HW, G * CIN], F8, kind="ExternalInput")
    xin = nc.dram_tensor("xin", [X_ROWS, HW], I8, kind="ExternalInput")
    # per-x-row int8 scales, laid out like attT: qrow[p, b*2+ch]
    qrow = nc.dram_tensor("qrow", [128, X_TILES], F32, kind="ExternalInput")
    # host-prepared weight layouts (see kernel() below)
    # w1t row 64 carries b1 (bias folded in via pooled's constant-1 row)
    w1t = nc.dram_tensor("w1t", [CIN + 1, G * HID], F8, kind="ExternalInput")
    w2t = nc.dram_tensor("w2t", [128, G * 4 * NCH], F8, kind="ExternalInput")
    b2r = nc.dram_tensor("b2r", [1, G * NCH], F32, kind="ExternalInput")
    out = nc.dram_tensor("out", [X_ROWS, HW], BF16, kind="ExternalOutput")

    (i128_np, sel4_np, selq_np, sel4t_np, eps16_np, one4_np,
     onesf8_np, one1b_np, allsel_np, allselq_np) = _consts()
    i128_d = nc.inline_tensor(i128_np, "ident128")
    sel4_d = nc.inline_tensor(sel4_np, "sel4")
    selq_d = nc.inline_tensor(selq_np, "selq")
    sel4t_d = nc.inline_tensor(sel4t_np, "sel4t")
    eps16_d = nc.inline_tensor(eps16_np, "eps16")
    one4_d = nc.inline_tensor(one4_np, "one4")
    onesf8_d = nc.inline_tensor(onesf8_np, "onesf8")
    one1b_d = nc.inline_tensor(one1b_np, "one1b")
    allsel_d = nc.inline_tensor(allsel_np, "allsel")
    allselq_d = nc.inline_tensor(allselq_np, "allselq")

    with tile.TileContext(nc) as tc:
        with (
            tc.tile_pool(name="consts", bufs=1) as cp,
            tc.tile_pool(name="stats", bufs=1) as sp,
            tc.tile_pool(name="embp", bufs=emb_bufs) as embp,
            tc.tile_pool(name="embtp", bufs=8) as embtp,
            tc.tile_pool(name="xp", bufs=x_bufs) as xp,
            tc.tile_pool(name="xop", bufs=4) as xop,
            tc.tile_pool(name="scratch", bufs=4) as scr,
            tc.tile_pool(name="psR", bufs=1, space="PSUM") as psR,
            tc.tile_pool(name="psA", bufs=1, space="PSUM") as psA,
            tc.tile_pool(name="psB", bufs=1, space="PSUM") as psB,
            tc.tile_pool(name="psC", bufs=2, space="PSUM") as psC,
            tc.tile_pool(name="psD", bufs=1, space="PSUM") as psD,
        ):
            # ---- constants / weights into SBUF ------------------------
            # Small consts go on the idle Pool queue; w1t early on ACT's
            # queue; w2t (2MB) is held until the embedding stream is done
            # so the spatial sums are not delayed.
            w1t_sb = cp.tile([CIN + 1, G * HID], F8, tag="w1t")
            w2t_sb = cp.tile([128, G * 4 * NCH], F8, tag="w2t")
            b2r_sb = cp.tile([1, G * NCH], F32, tag="b2r")
            i128_sb = cp.tile([128, 128], F32, tag="i128")
            sel4_sb = cp.tile([G * 4, 4], F32, tag="sel4")
            selq_sb = cp.tile([G * 4, 4], F32, tag="selq")
            sel4t_sb = cp.tile([4, G * 4], F32, tag="sel4t")
            eps16_sb = cp.tile([CAPS * B_LOC, 1], F32, tag="eps16")
            one4_sb = cp.tile([1, 4], F32, tag="one4")
            onesf8_sb = cp.tile([128, 1], F8, tag="onesf8")
            one1b_sb = cp.tile([1, 1], BF16, tag="one1b")
            allsel_sb = cp.tile([G * 4, G * 4], F32, tag="allsel")
            allselq_sb = cp.tile([G * 4, G * 4], F32, tag="allselq")
            nc.gpsimd.dma_start(onesf8_sb[:], onesf8_d[:])
            nc.gpsimd.dma_start(i128_sb[:], i128_d[:])
            nc.gpsimd.dma_start(one1b_sb[:], one1b_d[:])
            nc.gpsimd.dma_start(b2r_sb[:], b2r[:])
            nc.gpsimd.dma_start(one4_sb[:], one4_d[:])
            nc.gpsimd.dma_start(sel4_sb[:], sel4_d[:])
            nc.gpsimd.dma_start(selq_sb[:], selq_d[:])
            nc.gpsimd.dma_start(sel4t_sb[:], sel4t_d[:])
            nc.gpsimd.dma_start(eps16_sb[:], eps16_d[:])
            nc.gpsimd.dma_start(allsel_sb[:], allsel_d[:])
            nc.gpsimd.dma_start(allselq_sb[:], allselq_d[:])
            qrow_sb = cp.tile([128, X_TILES], F32, tag="qrow")
            nc.gpsimd.dma_start(qrow_sb[:], qrow[:])
            nc.scalar.dma_start(w1t_sb[:], w1t[:])

            # ---- phase 1: stream embedding, 3-engine spatial sums -----
            # Channel-major tiles (samples 0-2): two half-width reduces on
            # DVE/ACT each. Transposed tiles (sample 3): a ones-matmul on
            # PE accumulating into one [1, 512] psum chunk. Interleaved so
            # all three engines drain the stream concurrently.
            red_scr = sp.tile([128, TW // 2], BF16, tag="redscr")
            sums2 = sp.tile([128, 2 * N_TILES], F32, tag="sums")
            sums_w = sp.tile([128, B_LOC * 4], F32, tag="sumsw")
            pooled_cat = sp.tile([CIN + 1, G * B_LOC], F8, tag="pooled")
            nc.vector.memset(pooled_cat[CIN : CIN + 1, :], 1.0)
            pview = pooled_cat[0:CIN, :].rearrange("p (j r) -> p j r", r=8)
            rchunk = psR.tile([1, G * CIN], F32, tag="rchunk")
            emb_loads = []
            ti = 0

            def t_burst(n):
                # one packed tile = T_PACK spatial 128-blocks side by side
                nonlocal ti
                for _ in range(n):
                    if ti >= T_TILES:
                        return
                    tt = embtp.tile([128, T_PACK * G * CIN], F8, tag="embt")
                    src_v = emb_t[
                        ti * T_PACK * 128 : (ti + 1) * T_PACK * 128, :
                    ].rearrange("(k p) c -> p k c", p=128)
                    tld = nc.sync.dma_start(
                        tt[:].rearrange("p (k c) -> p k c", k=T_PACK), src_v
                    )
                    emb_loads.append(tld)
                    for k in range(T_PACK):
                        nc.tensor.matmul(
                            rchunk[:],
                            onesf8_sb[:],
                            tt[:, k * G * CIN : (k + 1) * G * CIN],
                            start=(ti == 0 and k == 0),
                            stop=(ti == T_TILES - 1 and k == T_PACK - 1),
                        )
                    ti += 1

            for t in range(N_TILES):
                et = embp.tile([128, TW], F8, tag="emb")
                ld = nc.sync.dma_start(et[:], emb_n[bass.ts(t, 128), :])
                emb_loads.append(ld)
                for hh in range(2):
                    eng = RED_ASSIGN[2 * t + hh]
                    ehalf = et[:, hh * (TW // 2) : (hh + 1) * (TW // 2)]
                    scol = sums2[:, 2 * t + hh : 2 * t + hh + 1]
                    if eng == "v":
                        last_v_red = nc.vector.reduce_sum(
                            scol, ehalf, axis=AX.X
                        )
                    else:
                        last_a_red = nc.scalar.activation(
                            red_scr[:], ehalf, AF.Identity, accum_out=scol
                        )
                t_burst((2, 2, 1, 1, 1, 1, 0, 0, 0, 0, 0, 0)[t])
                if t % 4 == 3:
                    # sample b's half-sums fold on the otherwise-idle Pool
                    # engine (SBUF only) while later tiles still stream
                    b = t // 4
                    s2v = sums2[:, 8 * b : 8 * b + 8].rearrange(
                        "p (j hh) -> p j hh", hh=2
                    )
                    nc.gpsimd.tensor_add(
                        sums_w[:, 4 * b : 4 * b + 4], s2v[:, :, 0], s2v[:, :, 1]
                    )
            t_burst(T_TILES - ti)

            # shift channels onto partitions; the DVE-side psum copies are
            # pinned behind DVE's last reduce so the Tile scheduler cannot
            # head-of-line-block the reduce queue with PE-dependent ops
            for b in range(B_N):
                for q in range(2):
                    pq = psC.tile([CIN, 4], F32, tag="pq")
                    nc.tensor.matmul(
                        pq[:],
                        i128_sb[:, q * 64 : (q + 1) * 64],
                        sums_w[:, 4 * b : 4 * b + 4],
                        start=True,
                        stop=True,
                    )
                    cpv = nc.vector.tensor_scalar_mul(
                        pview[:, :, 4 * q + b], pq[:], 1.0 / 64.0
                    )
                    tile.add_dep_helper(
                        cpv.ins, last_v_red.ins, sync=True,
                        reason="pooled copies run after the DVE reduces",
                    )

            # sample 3 pooled: psum row -> SBUF, channels back onto
            # partitions via K=1 matmuls, group pairs shifted with I128.
            row_sb = sp.tile([1, G * CIN], BF16, tag="prow")
            rsc = nc.scalar.activation(row_sb[:], rchunk[:], AF.Identity)
            tile.add_dep_helper(
                rsc.ins, last_a_red.ins, sync=True,
                reason="sample-3 row copy runs after the ACT reduces",
            )
            tp = psC.tile([128, 16], F32, tag="small")
            for m in range(4):
                nc.tensor.matmul(
                    tp[:, m : m + 1],
                    row_sb[0:1, m * 128 : (m + 1) * 128],
                    one1b_sb[:],
                    start=True,
                    stop=True,
                )
            tps = sp.tile([128, 4], F32, tag="tps")
            tcv = nc.vector.tensor_copy(tps[:], tp[:, 0:4])
            tile.add_dep_helper(
                tcv.ins, last_v_red.ins, sync=True,
                reason="sample-3 tps copy runs after the DVE reduces",
            )
            # tps[u, m] = pooled(b=3, ch m*128+u); group g = 2m + u//64
            pviews3 = pooled_cat[0:CIN, :].rearrange(
                "p (m Q r) -> p Q r m", Q=2, r=B_LOC
            )
            for q in range(2):
                pch = psC.tile([128, 16], F32, tag="small")
                nc.tensor.matmul(
                    pch[0:CIN, 0:4],
                    i128_sb[:, q * 64 : (q + 1) * 64],
                    tps[:],
                    start=True,
                    stop=True,
                )
                nc.vector.tensor_scalar_mul(
                    pviews3[:, q, B_LOC - 1], pch[0:CIN, 0:4], 1.0 / 64.0
                )
            # w2t transfer yields DMA bandwidth to the embedding stream;
            # hanging it off tile 12 lets its descriptor-gen overlap the
            # last tiles so the transfer starts the moment the stream ends
            w2t_ld = nc.scalar.dma_start(w2t_sb[:], w2t[:])
            tile.add_dep_helper(
                w2t_ld.ins, emb_loads[17].ins, sync=True,
                reason="w2t load yields DMA BW to embedding",
            )

            # ---- phase 2b: squeeze MLP, batched over 4 samples --------
            # All 32 h-chunk matmuls accumulate into ONE [128, 128] psum
            # tile (disjoint 4-col groups, bias via pooled's 1-row), then a
            # single Relu activation produces h. Matmuls run back-to-back
            # on PE with no per-chunk consumers.
            h_ps = psA.tile([128, G * 4 * B_LOC], F32, tag="hps")
            for g in range(G):
                for j in range(4):
                    c0 = (g * 4 + j) * 4
                    nc.tensor.matmul(
                        h_ps[:, c0 : c0 + 4],
                        w1t_sb[:, g * HID + j * 128 : g * HID + (j + 1) * 128],
                        pooled_cat[:, g * 4 : (g + 1) * 4],
                        start=True,
                        stop=True,
                    )
            h_cat = sp.tile([128, G * 4 * B_LOC], F8, tag="hcat")
            nc.scalar.activation(h_cat[:], h_ps[:], AF.Relu)

            # atts[:, (mc*8+g)*4+b] = w2 @ h + b2; bias joins each psum
            # accumulation group as a 5th matmul (b2 row x ones)
            a_ps = psA.tile([128, G * 4 * B_LOC], F32, tag="hps")
            for g in range(G):
                for mc in range(2):
                    c0 = (mc * 8 + g) * 4
                    for kc in range(4):
                        nc.tensor.matmul(
                            a_ps[:, c0 : c0 + 4],
                            w2t_sb[
                                :,
                                g * 4 * NCH + kc * NCH + mc * 128 : g * 4 * NCH
                                + kc * NCH
                                + mc * 128
                                + 128,
                            ],
                            h_cat[:, (g * 4 + kc) * 4 : (g * 4 + kc) * 4 + 4],
                            start=(kc == 0),
                            stop=False,
                        )
                    nc.tensor.matmul(
                        a_ps[:, c0 : c0 + 4],
                        b2r_sb[0:1, g * NCH + mc * 128 : g * NCH + (mc + 1) * 128],
                        one4_sb[:],
                        start=False,
                        stop=True,
                    )
            atts_all = sp.tile([128, 2 * G * B_LOC], F32, tag="atts")
            nc.vector.tensor_scalar_mul(
                atts_all[:], a_ps[:, 0 : 2 * G * B_LOC], 1.0 / 64.0
            )

            # ---- phase 2c: transpose -> xr4 [32, 256], row g*4+b ------
            xr4 = sp.tile([G * B_LOC, NCH], F32, tag="xr4")
            av = atts_all[:].rearrange("p (m c) -> p m c", m=2)
            for mc in range(2):
                if mc == 0:
                    pt = psB.tile([G * B_LOC, 128], F32, tag="route")
                else:
                    pt = psC.tile([G * B_LOC, 128], F32, tag="small")
                nc.tensor.transpose(pt[:], av[:, mc], i128_sb[:])
                nc.vector.tensor_copy(
                    xr4[:, mc * 128 : (mc + 1) * 128], pt[:]
                )

            # ---- phase 2d: dynamic routing, all 4 samples -------------
            # beta[g*4+b, cap]. Each norm iteration runs two parallel
            # branches off wxr: (A) v = per-sample group sum -> squared
            # capsule norms -> rsqrt (ACT-heavy), and (B) the UNnormalized
            # increment binc_raw = sum_o v*xr via an all-sample broadcast
            # matmul + fused multiply-reduce (PE/DVE). rsqrt lands as a
            # tiny post-scale, so branch A's latency hides behind B.
            beta = sp.tile([G * B_LOC, CAPS], F32, tag="beta")
            att4 = sp.tile([B_LOC, NCH], F32, tag="att4")
            for it in range(iters):
                if it == 0:
                    wsrc = xr4
                    wsel, wallsel = selq_sb, allselq_sb
                else:
                    # beta stays small (|beta| < ~3); skip max-shift
                    e = sp.tile([G * B_LOC, CAPS], F32, tag="e")
                    s = sp.tile([G * B_LOC, 1], F32, tag="s")
                    nc.scalar.activation(e[:], beta[:], AF.Exp, accum_out=s[:])
                    rs = sp.tile([G * B_LOC, 1], F32, tag="rs")
                    nc.vector.reciprocal(rs[:], s[:])
                    # wxr = (xr * rs) * e_bcast  (one fused DVE op)
                    wxr = scr.tile([G * B_LOC, NCH], F32, tag="rt", name=f"wxr{it}")
                    e3 = e[:].rearrange("p (c u) -> p c u", u=1)
                    nc.vector.scalar_tensor_tensor(
                        wxr[:].rearrange("p (c o) -> p c o", o=OUT),
                        xr4[:].rearrange("p (c o) -> p c o", o=OUT),
                        rs[:],
                        e3.broadcast_to([G * B_LOC, CAPS, OUT]),
                        ALU.mult,
                        ALU.mult,
                    )
                    wsrc = wxr
                    wsel, wallsel = sel4_sb, allsel_sb
                if it == iters - 1:
                    vp = psB.tile([G * B_LOC, NCH], F32, tag="route")
                    nc.tensor.matmul(
                        vp[0:B_LOC, :], wsel[:], wsrc[:], start=True, stop=True
                    )
                    # sigmoid(x) = 1/(1+exp(-x))
                    eneg = scr.tile([B_LOC, NCH], F32, tag="rt1", name="eneg")
                    nc.scalar.activation(
                        eneg[:], vp[0:B_LOC, :], AF.Exp, scale=-1.0
                    )
                    # att4 holds 1+exp(-v); the (cheap, per-ch [128,4])
                    # reciprocal happens after the transposes below
                    nc.vector.tensor_scalar_add(att4[:], eneg[:], 1.0)
                else:
                    # one matmul serves both branches: bc[g*4+b,:] = v[b,:]
                    # for every g, so rows 0:4 (the g=0 block) ARE v — the
                    # separate per-sample vp matmul is redundant
                    bc = psB.tile([G * B_LOC, NCH], F32, tag="route")
                    nc.tensor.matmul(
                        bc[:], wallsel[:], wsrc[:], start=True, stop=True
                    )
                    # branch A: capsule norms -> rn = rsqrt(n2)
                    sq = scr.tile([B_LOC, NCH], F32, tag="rt1", name=f"sq{it}")
                    nc.scalar.square(sq[:], bc[0:B_LOC, :])
                    n2 = sp.tile([B_LOC, CAPS], F32, tag="n2")
                    nc.vector.reduce_sum(
                        n2[:],
                        sq[:].rearrange("p (c o) -> p c o", o=OUT),
                        axis=AX.X,
                    )
                    lnn = sp.tile([B_LOC, CAPS], F32, tag="lnn")
                    nc.scalar.activation(
                        lnn[:], n2[:], AF.Ln, bias=eps16_sb[0:B_LOC, :]
                    )
                    rn = sp.tile([B_LOC, CAPS], F32, tag="rn")
                    nc.scalar.activation(rn[:], lnn[:], AF.Exp, scale=-0.5)
                    prod = scr.tile([G * B_LOC, NCH], F32, tag="rt", name=f"pr{it}")
                    nc.vector.tensor_mul(prod[:], bc[:], xr4[:])
                    braw = sp.tile([G * B_LOC, CAPS], F32, tag=f"braw{it}")
                    nc.vector.reduce_sum(
                        braw[:],
                        prod[:].rearrange("p (c o) -> p c o", o=OUT),
                        axis=AX.X,
                    )
                    # join: scale by rn (broadcast to g rows via sel4t)
                    rnx = psD.tile([G * B_LOC, CAPS], F32, tag="rnx")
                    nc.tensor.matmul(
                        rnx[:], sel4t_sb[:], rn[:], start=True, stop=True
                    )
                    if it == 0:
                        nc.vector.tensor_mul(beta[:], braw[:], rnx[:])
                    else:
                        binc = sp.tile([G * B_LOC, CAPS], F32, tag="binc")
                        nc.vector.tensor_mul(binc[:], braw[:], rnx[:])
                        nc.vector.tensor_add(beta[:], beta[:], binc[:])

            # ---- phase 2e: att4 [4, 256] -> attT [128, 8] col b*2+ch --
            attT = sp.tile([128, X_TILES], F32, tag="attT")
            attq = sp.tile([128, X_TILES], F32, tag="attq")
            atqv = attq[:].rearrange("p (b c) -> p b c", c=2)
            qv = qrow_sb[:].rearrange("p (b c) -> p b c", c=2)
            atv = attT[:].rearrange("p (b c) -> p b c", c=2)
            for ch in range(2):
                pt2 = psC.tile([128, 16], F32, tag="small")
                nc.tensor.transpose(
                    pt2[:, 0:B_LOC],
                    att4[:, ch * 128 : (ch + 1) * 128],
                    i128_sb[0:B_LOC, 0:B_LOC],
                )
                # reciprocal straight from the psum transpose output
                # (skips a separate psum->sbuf copy on the critical path)
                nc.vector.reciprocal(atv[:, :, ch], pt2[:, 0:B_LOC])
                nc.vector.tensor_mul(atqv[:, :, ch], atv[:, :, ch], qv[:, :, ch])
            # int8 row scales folded per ch-half (emitted in the loop above)

            # ---- phase 3: scale x (int8 in, bf16 out) -----------------
            # x row = b*256 + ch2 ; row block r: b = r//2, ch = r%2.
            # The whole store stream is gated by the first finished scale,
            # so scale+store run on sub-tiles: quarters for the first two
            # tiles (fast first store), halves after, alternating ACT/DVE.
            for r in range(X_TILES):
                xt = xp.tile([128, TW], I8, tag="x")
                xo = xop.tile([128, TW], BF16, tag="xo")
                xld = nc.sync.dma_start(xt[:], xin[bass.ts(r, 128), :])
                # park x loads behind tile 13 so they reach the DMA engines
                # after w2t but with their descriptor-gen already done
                tile.add_dep_helper(
                    xld.ins, emb_loads[18].ins, sync=True,
                    reason="x loads yield DMA BW to embedding + w2t",
                )
                nsub = 4 if r == 0 else 1
                sw = TW // nsub
                for k in range(nsub):
                    xts = xt[:, k * sw : (k + 1) * sw]
                    xos = xo[:, k * sw : (k + 1) * sw]
                    if (r + k) % 2 == 0:
                        nc.scalar.activation(
                            xos, xts, AF.Identity, scale=attq[:, r : r + 1]
                        )
                    else:
                        nc.vector.tensor_scalar_mul(
                            xos, xts, attq[:, r : r + 1]
                        )
                    nc.scalar.dma_start(
                        out[bass.ts(r, 128), k * sw : (k + 1) * sw], xos
                    )

    nc.compile()
    return nc


def _prep_weights(w1, b1, w2, b2):
    w1 = np.asarray(w1, dtype=np.float32)
    b1 = np.asarray(b1, dtype=np.float32)
    w2 = np.asarray(w2, dtype=np.float32)
    b2 = np.asarray(b2, dtype=np.float32)
    import ml_dtypes

    # Scaling: pooled streams through fp8 as 64*mean (the 1/HW/64 factor
    # is applied by the pooled psum copies), so the MLP runs at x64 scale:
    # w1t rows hold raw w1, the bias row holds 64*b1, b2r holds 64*b2, and
    # the atts copy divides by 64. Everything stays in fp8's normal range.
    # w1t[i, g*512+o] = w1[g, o, i]; row 64 = 64*b1[g, o]
    w1t = np.concatenate(
        [
            w1.transpose(2, 0, 1).reshape(CIN, G * HID),
            64.0 * b1.reshape(1, G * HID),
        ],
        axis=0,
    ).astype(ml_dtypes.float8_e4m3)
    w1t = np.ascontiguousarray(w1t)
    # w2t[p, g*1024 + kc*256 + o2] = w2[g, o2, kc*128+p]
    w2t = np.ascontiguousarray(
        w2.transpose(0, 2, 1)
        .reshape(G, 4, 128, NCH)
        .transpose(2, 0, 1, 3)
        .reshape(128, G * 4 * NCH)
        .astype(ml_dtypes.float8_e4m3)
    )
    # b2r[0, g*256+c] = 64 * b2[g, c]
    b2r = np.ascontiguousarray(64.0 * b2.reshape(1, G * NCH))
    return w1t, w2t, b2r


def make_in_maps(embedding, x, w1, b1, w2, b2):
    import ml_dtypes

    embedding = np.asarray(embedding)
    x = np.asarray(x)
    w1t, w2t, b2r = _prep_weights(w1, b1, w2, b2)
    in_maps = []
    for c in range(N_CORES):
        xc = np.ascontiguousarray(
            x[c * B_LOC : (c + 1) * B_LOC], dtype=np.float32
        ).reshape(X_ROWS, HW)
        # per-row int8 quantization; scales fold into the attention scalars
        q = np.abs(xc).max(axis=1, keepdims=True) / 127.0
        xi8 = np.clip(np.round(xc / q), -127, 127).astype(np.int8)
        # qrow[p, b*2+ch] = q of x row (b*2+ch)*128+p
        qr = np.ascontiguousarray(
            q.reshape(X_TILES, 128).transpose(1, 0).astype(np.float32)
        )
        in_maps.append(
            {
                "emb_n": np.ascontiguousarray(
                    embedding[c * B_LOC : c * B_LOC + B_N]
                )
                .reshape(N_ROWS, HW)
                .astype(ml_dtypes.float8_e4m3),
                "emb_t": np.ascontiguousarray(
                    embedding[c * B_LOC + B_N]
                    .reshape(G * CIN, HW)
                    .transpose(1, 0)
                )
                .astype(ml_dtypes.float8_e4m3),
                "xin": xi8,
                "qrow": qr,
                "w1t": w1t,
                "w2t": w2t,
                "b2r": b2r,
            }
        )
    return in_maps


def kernel(embedding, x, w1, b1, w2, b2):
    # This axon client has no NTFF profiling hook; a stray BASS_TRACE in the
    # environment would crash run_bass_kernel_spmd's trace path.
    os.environ.setdefault("BASS_NEVER_TRACE", "1")
    nc = build_nc()
    in_maps = make_in_maps(embedding, x, w1, b1, w2, b2)
    res = run_bass_kernel_spmd(nc, in_maps, core_ids=list(range(N_CORES)))
    out = np.concatenate(
        [
            np.asarray(r["out"]).astype(np.float32).reshape(B_LOC, NCH, 64, 64)
            for r in res.results
        ],
        axis=0,
    )
    return out
